# revision 29
# baseline (speedup 1.0000x reference)
"""Trainium2 Bass kernel for nn_Compressor (sparse-attention KV compressor).

Pipeline (data-parallel over 8 NeuronCores, 4096 tokens / 1024 windows each):
  kv_score = x @ W.T (bf16 x bf16 -> fp32)        [tokens, 1024]
  g0/g1    = kv * sigmoid(gate)                    gated chunks, fp32
  comp     = overlapping-window compression        [windows, 256]
  y        = RoPE(RMSNorm(comp))                   [windows, 256]
  out      = kv_buffer with rows[loc] = y          [65536, 256]

Sharding: x is split along the token axis (4096 tokens/core + the 4
boundary tokens of the previous shard for the overlapping chunk); W/ape/
norm_weight are replicated; each core returns its 1024 compressed rows
and the host scatters them into the pool at `loc` (the unshard step).

Device-side layout: x and W are staged transposed ([DIM, tokens] /
[DIM, 1024]) so the matmul runs with the contraction dim on SBUF
partitions and produces kv_score^T tiles [128 features, 456 tokens]
directly. Gating, windowed compression (strided-AP reads, per-partition
ape scalars) and a per-token-tile compression chunk all stay in that
[d, token] layout; only the final [d, w] -> [w, d] flip of the small
compressed output goes through PE transposes. Compression is emitted
per token tile so it pipelines behind the matmul stream instead of
serializing at the end; g0 tiles carry a 4-column spill copied from the
next tile so the stride-4 window reads stay inside one allocation.
"""

import sys

if "/opt/trn_rl_repo" not in sys.path:  # already present under axon boot
    sys.path.insert(0, "/opt/trn_rl_repo")

import numpy as np
import ml_dtypes

import concourse.bass as bass
import concourse.mybir as mybir
import concourse.tile as tile
from concourse import bacc
from concourse.bass_utils import run_bass_kernel_spmd
from concourse.masks import make_identity

BF16 = ml_dtypes.bfloat16
F32 = mybir.dt.float32
DT_BF16 = mybir.dt.bfloat16
AF = mybir.ActivationFunctionType
OP = mybir.AluOpType

P = 128          # SBUF partitions
HD = 256         # compressed head dim
RATIO = 4        # compression stride
ROPE = 64        # rotary dims
EPS = 1e-6
N_CORES = 8

# Full-problem sizes
T = 32768
DIM = 4096
TOK = T // N_CORES        # tokens per core
NW = TOK // RATIO         # windows (compressed tokens) per core
NTOK = TOK + 2 * RATIO    # + 4 boundary tokens in front, + 4 zero pad tail
FREE = 456                # matmul moving free dim; NTOK = 9 * 456
POOL = 65536


class Cfg:
    def __init__(self, dim=DIM, ntok=NTOK, free=FREE, nw=NW, x_bufs=64):
        assert ntok % free == 0
        assert free % RATIO == 0
        assert nw % P == 0
        assert ntok == 4 * nw + 2 * RATIO
        self.dim = dim
        self.ntok = ntok
        self.free = free
        self.nt = ntok // free
        self.kt = dim // P
        self.nw = nw
        self.wt_tiles = nw // P
        self.x_bufs = x_bufs
        self.chunk_nw = free // RATIO            # windows per compression chunk
        self.last_nw = nw - (self.nt - 1) * self.chunk_nw


FULL = Cfg()


def build_nc(cfg=FULL, num_devices=N_CORES):
    nc = bacc.Bacc(
        "TRN2", target_bir_lowering=False, debug=False, num_devices=num_devices
    )
    xs_d = nc.dram_tensor("xs", [cfg.dim, cfg.ntok], DT_BF16, kind="ExternalInput")
    wt_d = nc.dram_tensor("wt", [cfg.dim, 4 * HD], DT_BF16, kind="ExternalInput")
    ape_d = nc.dram_tensor("ape", [P, 16], F32, kind="ExternalInput")
    nw_d = nc.dram_tensor("nw", [P, HD], F32, kind="ExternalInput")
    cos_d = nc.dram_tensor(
        "cosp", [P, cfg.wt_tiles, ROPE // 2], F32, kind="ExternalInput"
    )
    sin_d = nc.dram_tensor(
        "sinp", [P, cfg.wt_tiles, ROPE // 2], F32, kind="ExternalInput"
    )
    y_d = nc.dram_tensor("y", [cfg.nw, HD], F32, kind="ExternalOutput")

    with tile.TileContext(nc) as tc:
        with (
            tc.tile_pool(name="consts", bufs=1) as consts,
            tc.tile_pool(name="wts", bufs=1) as wts,
            tc.tile_pool(name="xp", bufs=cfg.x_bufs) as xp,
            tc.tile_pool(name="gp", bufs=3) as gp,
            tc.tile_pool(name="cb", bufs=1) as cb,
            tc.tile_pool(name="work", bufs=3) as work,
            tc.tile_pool(name="mm", bufs=4, space="PSUM") as mmp,
            tc.tile_pool(name="tp", bufs=4, space="PSUM") as tpp,
        ):
            # ---- constants ----
            ape_t = consts.tile([P, 16], F32, tag="ape")
            nc.sync.dma_start(out=ape_t, in_=ape_d[:, :])
            nw_t = consts.tile([P, HD], F32, tag="nw")
            nc.sync.dma_start(out=nw_t, in_=nw_d[:, :])
            cos_t = consts.tile([P, cfg.wt_tiles, ROPE // 2], F32, tag="cos")
            nc.sync.dma_start(out=cos_t, in_=cos_d[:, :, :])
            sin_t = consts.tile([P, cfg.wt_tiles, ROPE // 2], F32, tag="sin")
            nc.sync.dma_start(out=sin_t, in_=sin_d[:, :, :])
            ident = consts.tile([P, P], F32, tag="id")
            make_identity(nc, ident)
            eps_t = consts.tile([P, 1], F32, tag="eps")
            nc.vector.memset(eps_t, EPS)

            # PE pre-warm: dummy matmuls on the identity keep the clock
            # ramping while the first x/W DMAs are still in flight
            warm = tpp.tile([P, P], F32, tag="tp", name="warm")
            for _ in range(40):
                nc.tensor.matmul(warm, ident[:, 0:P], ident, start=True, stop=True)

            # first token tile's x loads go out before W so PE can start as
            # soon as weights land; W is split kv-half then gate-half so the
            # first pair's kv matmuls wait on only half the weight bytes
            xts0 = []
            for k in range(cfg.kt):
                xt = xp.tile([P, cfg.free], DT_BF16, tag="xt", name=f"x0_{k}")
                nc.sync.dma_start(out=xt, in_=xs_d[k * P : (k + 1) * P, 0 : cfg.free])
                xts0.append(xt)
            KG = 4  # k-chunks per W DMA
            wt_kv = wts.tile([P, cfg.kt, 2 * HD], DT_BF16, tag="wkv", name="wt_kv")
            wt_gt = wts.tile([P, cfg.kt, 2 * HD], DT_BF16, tag="wgt", name="wt_gt")
            for half, dst in ((0, wt_kv), (1, wt_gt)):
                c0 = half * 2 * HD
                for g in range(cfg.kt // KG):
                    nc.sync.dma_start(
                        out=dst[:, g * KG : (g + 1) * KG, :],
                        in_=wt_d[
                            g * KG * P : (g + 1) * KG * P, c0 : c0 + 2 * HD
                        ].rearrange("(k p) c -> p k c", p=P),
                    )

            def w_ap(f, k):
                if f < 4:
                    return wt_kv[:, k, f * P : (f + 1) * P]
                return wt_gt[:, k, (f - 4) * P : (f - 3) * P]

            # comp junction buffers, [d-half partitions, windows]
            comp = [
                cb.tile([P, cfg.nw], F32, tag=f"comp{h}", name=f"comp{h}")
                for h in range(2)
            ]

            # feature tiles: 0,1 = kv chunk0 | 2,3 = kv chunk1
            #                4,5 = gate chunk0 | 6,7 = gate chunk1
            pairs = [(0, 4, "g0a"), (1, 5, "g0b"), (2, 6, "g1a"), (3, 7, "g1b")]
            g_tiles = {nm: {} for _, _, nm in pairs}

            def run_pair(j, fkv, fg, gname):
                pk = mmp.tile([P, cfg.free], F32, tag="mm", name=f"pk{j}_{fkv}")
                for k in range(cfg.kt):
                    nc.tensor.matmul(
                        pk,
                        w_ap(fkv, k),
                        xts[k],
                        start=(k == 0),
                        stop=(k == cfg.kt - 1),
                    )
                pg = mmp.tile([P, cfg.free], F32, tag="mm", name=f"pg{j}_{fg}")
                for k in range(cfg.kt):
                    nc.tensor.matmul(
                        pg,
                        w_ap(fg, k),
                        xts[k],
                        start=(k == 0),
                        stop=(k == cfg.kt - 1),
                    )
                sg = work.tile([P, cfg.free], F32, tag="sig", name=f"sg{j}_{fg}")
                nc.scalar.activation(out=sg, in_=pg, func=AF.Sigmoid)
                # g0 tiles carry a 4-col spill at the end (next tile's first
                # 4 tokens); g1 window reads never cross the tile boundary.
                width = cfg.free + 4 if gname.startswith("g0") else cfg.free
                gt = gp.tile([P, width], F32, tag=gname, name=f"{gname}_{j}")
                nc.vector.tensor_mul(out=gt[:, 0 : cfg.free], in0=pk, in1=sg)
                g_tiles[gname][j] = gt

            def emit_comp_chunk(j, part=None):
                nwin = cfg.chunk_nw if j < cfg.nt - 1 else cfg.last_nw
                col0 = cfg.chunk_nw * j
                for h in range(2):
                    acc = comp[h][:, col0 : col0 + nwin]
                    if part in (None, "g0"):
                        g0 = g_tiles["g0a" if h == 0 else "g0b"][j]
                        for r in range(RATIO):
                            v0 = g0[:, 4 + r : 4 + r + 4 * (nwin - 1) + 1 : 4]
                            s0 = ape_t[:, r * 2 + h : r * 2 + h + 1]
                            if r == 0:
                                nc.vector.tensor_scalar_mul(
                                    out=acc, in0=v0, scalar1=s0
                                )
                            else:
                                nc.vector.scalar_tensor_tensor(
                                    out=acc, in0=v0, scalar=s0, in1=acc,
                                    op0=OP.mult, op1=OP.add,
                                )
                    if part in (None, "g1"):
                        g1 = g_tiles["g1a" if h == 0 else "g1b"][j]
                        for r in range(RATIO):
                            v1 = g1[:, r : r + 4 * (nwin - 1) + 1 : 4]
                            s1 = ape_t[:, 8 + r * 2 + h : 8 + r * 2 + h + 1]
                            nc.vector.scalar_tensor_tensor(
                                out=acc, in0=v1, scalar=s1, in1=acc,
                                op0=OP.mult, op1=OP.add,
                            )

            # ---- phase C: transpose -> RMSNorm -> RoPE -> store ----
            def emit_out_tile(i):
                pta = tpp.tile([P, P], F32, tag="tp", name=f"pta{i}")
                nc.tensor.transpose(pta, comp[0][:, i * P : (i + 1) * P], ident)
                ptb = tpp.tile([P, P], F32, tag="tp", name=f"ptb{i}")
                nc.tensor.transpose(ptb, comp[1][:, i * P : (i + 1) * P], ident)
                yt = work.tile([P, HD], F32, tag="y", name=f"y{i}")
                nc.scalar.copy(out=yt[:, 0:P], in_=pta)
                nc.vector.tensor_copy(out=yt[:, P : 2 * P], in_=ptb)

                sq = work.tile([P, HD], F32, tag="sq", name=f"sq{i}")
                ss = work.tile([P, 1], F32, tag="ss", name=f"ss{i}")
                nc.scalar.activation(
                    out=sq, in_=yt, func=AF.Square, accum_out=ss
                )
                rstd = work.tile([P, 1], F32, tag="rstd", name=f"rstd{i}")
                nc.scalar.activation(
                    out=rstd, in_=ss, func=AF.Sqrt, scale=1.0 / HD, bias=eps_t
                )
                rms = work.tile([P, 1], F32, tag="rms", name=f"rms{i}")
                nc.vector.reciprocal(out=rms, in_=rstd)
                nc.vector.tensor_scalar_mul(out=yt, in0=yt, scalar1=rms)
                nc.vector.tensor_mul(out=yt, in0=yt, in1=nw_t)

                y1 = yt[:, 0:ROPE:2]
                y2 = yt[:, 1:ROPE:2]
                cc = cos_t[:, i, :]
                sn = sin_t[:, i, :]
                m1 = work.tile([P, ROPE // 2], F32, tag="m1", name=f"m1_{i}")
                nc.gpsimd.tensor_mul(out=m1, in0=y1, in1=cc)
                m2 = work.tile([P, ROPE // 2], F32, tag="m2", name=f"m2_{i}")
                nc.gpsimd.tensor_mul(out=m2, in0=y2, in1=sn)
                m3 = work.tile([P, ROPE // 2], F32, tag="m3", name=f"m3_{i}")
                nc.vector.tensor_mul(out=m3, in0=y1, in1=sn)
                m4 = work.tile([P, ROPE // 2], F32, tag="m4", name=f"m4_{i}")
                nc.vector.tensor_mul(out=m4, in0=y2, in1=cc)
                nc.vector.tensor_sub(out=yt[:, 0:ROPE:2], in0=m1, in1=m2)
                nc.vector.tensor_add(out=yt[:, 1:ROPE:2], in0=m3, in1=m4)

                nc.sync.dma_start(out=y_d[i * P : (i + 1) * P, :], in_=yt)

            # ---- phase A + B: projection, gating, windowed compression ----
            for j in range(cfg.nt):
                if j == 0:
                    xts = xts0
                else:
                    xts = []
                    for k in range(cfg.kt):
                        xt = xp.tile(
                            [P, cfg.free], DT_BF16, tag="xt", name=f"x{j}_{k}"
                        )
                        nc.sync.dma_start(
                            out=xt,
                            in_=xs_d[
                                k * P : (k + 1) * P,
                                j * cfg.free : (j + 1) * cfg.free,
                            ],
                        )
                        xts.append(xt)
                run_pair(j, *pairs[0])
                run_pair(j, *pairs[1])
                if j >= 1:
                    for nm in ("g0a", "g0b"):
                        nc.gpsimd.tensor_copy(
                            out=g_tiles[nm][j - 1][:, cfg.free : cfg.free + 4],
                            in_=g_tiles[nm][j][:, 0:4],
                        )
                    # chunk j-1 is fully determined here (its g1 tiles are
                    # from iteration j-1); runs on DVE under pairs 2/3 matmuls
                    emit_comp_chunk(j - 1)
                if j == cfg.nt - 1:
                    # last chunk: g0 part overlaps its own pairs-2/3 matmuls
                    emit_comp_chunk(j, part="g0")
                    # output tiles whose comp chunks are already complete run
                    # under the final pairs' matmuls
                    for i in range(max(0, cfg.wt_tiles - 2)):
                        emit_out_tile(i)
                run_pair(j, *pairs[2])
                run_pair(j, *pairs[3])
            emit_comp_chunk(cfg.nt - 1, part="g1")
            for i in range(max(0, cfg.wt_tiles - 2), cfg.wt_tiles):
                emit_out_tile(i)



    nc.compile()
    return nc


def make_core_inputs(cfg, x_tokens, W, ape, norm_weight, cos_shard, sin_shard):
    """Per-core in_map from natural-layout shards.

    x_tokens: [ntok, dim] bf16 (boundary + own tokens + tail pad, already
    assembled); W: [1024, dim] bf16; ape: [8, 256] f32;
    cos/sin_shard: [nw, 32] f32.
    """
    xs = np.ascontiguousarray(x_tokens.T.astype(BF16, copy=False))
    wt = np.ascontiguousarray(W.T.astype(BF16, copy=False))
    ape2 = np.asarray(ape, np.float32).reshape(2, RATIO, HD)
    ape_sc = np.empty((P, 16), np.float32)
    for c in range(2):
        for r in range(RATIO):
            for h in range(2):
                ape_sc[:, c * 8 + r * 2 + h] = ape2[c, r, h * P : (h + 1) * P]
    nw_b = np.ascontiguousarray(
        np.broadcast_to(np.asarray(norm_weight, np.float32), (P, HD))
    )
    cosp = np.ascontiguousarray(
        np.asarray(cos_shard, np.float32)
        .reshape(cfg.wt_tiles, P, ROPE // 2)
        .transpose(1, 0, 2)
    )
    sinp = np.ascontiguousarray(
        np.asarray(sin_shard, np.float32)
        .reshape(cfg.wt_tiles, P, ROPE // 2)
        .transpose(1, 0, 2)
    )
    return {
        "xs": xs, "wt": wt, "ape": ape_sc, "nw": nw_b, "cosp": cosp, "sinp": sinp
    }


_NC = None
last_results = None


def kernel(**inputs):
    global _NC, last_results
    x = np.asarray(inputs["x"])
    W = np.asarray(inputs["W"])
    ape = np.asarray(inputs["ape"], dtype=np.float32)
    norm_weight = np.asarray(inputs["norm_weight"], dtype=np.float32)
    cos = np.asarray(inputs["cos"], dtype=np.float32)
    sin = np.asarray(inputs["sin"], dtype=np.float32)
    kv_buffer = np.asarray(inputs["kv_buffer"], dtype=np.float32)
    loc = np.asarray(inputs["loc"]).astype(np.int64)

    cfg = FULL
    if _NC is None:
        _NC = build_nc(cfg)

    x_bf = x.astype(BF16, copy=False)
    in_maps = []
    for c in range(N_CORES):
        xtok = np.zeros((cfg.ntok, DIM), BF16)
        lo = TOK * c - RATIO
        s = max(lo, 0)
        e = TOK * c + TOK
        xtok[s - lo : s - lo + (e - s)] = x_bf[s:e]
        in_maps.append(
            make_core_inputs(
                cfg, xtok, W, ape, norm_weight,
                cos[c * NW : (c + 1) * NW], sin[c * NW : (c + 1) * NW],
            )
        )

    res = run_bass_kernel_spmd(_NC, in_maps, core_ids=list(range(N_CORES)))
    last_results = res

    y_full = np.concatenate([res.results[c]["y"] for c in range(N_CORES)], axis=0)
    out = kv_buffer.copy()
    out[loc] = y_full
    return out


# revision 30
# speedup vs baseline: 1.0311x; 1.0311x over previous
"""Trainium2 Bass kernel for nn_Compressor (sparse-attention KV compressor).

Pipeline (data-parallel over 8 NeuronCores, 4096 tokens / 1024 windows each):
  kv_score = x @ W.T (bf16 x bf16 -> fp32)        [tokens, 1024]
  g0/g1    = kv * sigmoid(gate)                    gated chunks, fp32
  comp     = overlapping-window compression        [windows, 256]
  y        = RoPE(RMSNorm(comp))                   [windows, 256]
  out      = kv_buffer with rows[loc] = y          [65536, 256]

Sharding: x is split along the token axis (4096 tokens/core + the 4
boundary tokens of the previous shard for the overlapping chunk); W/ape/
norm_weight are replicated; each core returns its 1024 compressed rows
and the host scatters them into the pool at `loc` (the unshard step).

Device-side layout: x and W are staged transposed ([DIM, tokens] /
[DIM, 1024]) so the matmul runs with the contraction dim on SBUF
partitions and produces kv_score^T tiles [128 features, 456 tokens]
directly. Gating, windowed compression (strided-AP reads, per-partition
ape scalars) and a per-token-tile compression chunk all stay in that
[d, token] layout; only the final [d, w] -> [w, d] flip of the small
compressed output goes through PE transposes. Compression is emitted
per token tile so it pipelines behind the matmul stream instead of
serializing at the end; g0 tiles carry a 4-column spill copied from the
next tile so the stride-4 window reads stay inside one allocation.
"""

import sys

if "/opt/trn_rl_repo" not in sys.path:  # already present under axon boot
    sys.path.insert(0, "/opt/trn_rl_repo")

import numpy as np
import ml_dtypes

import concourse.bass as bass
import concourse.mybir as mybir
import concourse.tile as tile
from concourse import bacc
from concourse.bass_utils import run_bass_kernel_spmd
from concourse.masks import make_identity

BF16 = ml_dtypes.bfloat16
F32 = mybir.dt.float32
DT_BF16 = mybir.dt.bfloat16
AF = mybir.ActivationFunctionType
OP = mybir.AluOpType

P = 128          # SBUF partitions
HD = 256         # compressed head dim
RATIO = 4        # compression stride
ROPE = 64        # rotary dims
EPS = 1e-6
N_CORES = 8

# Full-problem sizes
T = 32768
DIM = 4096
TOK = T // N_CORES        # tokens per core
NW = TOK // RATIO         # windows (compressed tokens) per core
NTOK = TOK + 2 * RATIO    # + 4 boundary tokens in front, + 4 zero pad tail
FREE = 456                # matmul moving free dim; NTOK = 9 * 456
POOL = 65536


class Cfg:
    def __init__(self, dim=DIM, ntok=NTOK, free=FREE, nw=NW, x_bufs=64):
        assert ntok % free == 0
        assert free % RATIO == 0
        assert nw % P == 0
        assert ntok == 4 * nw + 2 * RATIO
        self.dim = dim
        self.ntok = ntok
        self.free = free
        self.nt = ntok // free
        self.kt = dim // P
        self.nw = nw
        self.wt_tiles = nw // P
        self.x_bufs = x_bufs
        self.chunk_nw = free // RATIO            # windows per compression chunk
        self.last_nw = nw - (self.nt - 1) * self.chunk_nw


FULL = Cfg()


def build_nc(cfg=FULL, num_devices=N_CORES):
    nc = bacc.Bacc(
        "TRN2", target_bir_lowering=False, debug=False, num_devices=num_devices
    )
    xs_d = nc.dram_tensor("xs", [cfg.dim, cfg.ntok], DT_BF16, kind="ExternalInput")
    wt_d = nc.dram_tensor("wt", [cfg.dim, 4 * HD], DT_BF16, kind="ExternalInput")
    ape_d = nc.dram_tensor("ape", [P, 16], F32, kind="ExternalInput")
    nw_d = nc.dram_tensor("nw", [P, HD], F32, kind="ExternalInput")
    cos_d = nc.dram_tensor(
        "cosp", [P, cfg.wt_tiles, ROPE // 2], F32, kind="ExternalInput"
    )
    sin_d = nc.dram_tensor(
        "sinp", [P, cfg.wt_tiles, ROPE // 2], F32, kind="ExternalInput"
    )
    y_d = nc.dram_tensor("y", [cfg.nw, HD], F32, kind="ExternalOutput")

    with tile.TileContext(nc) as tc:
        with (
            tc.tile_pool(name="consts", bufs=1) as consts,
            tc.tile_pool(name="wts", bufs=1) as wts,
            tc.tile_pool(name="xp", bufs=cfg.x_bufs) as xp,
            tc.tile_pool(name="gp", bufs=3) as gp,
            tc.tile_pool(name="cb", bufs=1) as cb,
            tc.tile_pool(name="work", bufs=3) as work,
            tc.tile_pool(name="mm", bufs=4, space="PSUM") as mmp,
            tc.tile_pool(name="tp", bufs=4, space="PSUM") as tpp,
        ):
            # ---- constants ----
            ape_t = consts.tile([P, 16], F32, tag="ape")
            nc.sync.dma_start(out=ape_t, in_=ape_d[:, :])
            nw_t = consts.tile([P, HD], F32, tag="nw")
            nc.sync.dma_start(out=nw_t, in_=nw_d[:, :])
            cos_t = consts.tile([P, cfg.wt_tiles, ROPE // 2], F32, tag="cos")
            nc.sync.dma_start(out=cos_t, in_=cos_d[:, :, :])
            sin_t = consts.tile([P, cfg.wt_tiles, ROPE // 2], F32, tag="sin")
            nc.sync.dma_start(out=sin_t, in_=sin_d[:, :, :])
            ident = consts.tile([P, P], F32, tag="id")
            make_identity(nc, ident)
            eps_t = consts.tile([P, 1], F32, tag="eps")
            nc.vector.memset(eps_t, EPS)

            # PE pre-warm: dummy matmuls on the identity keep the clock
            # ramping while the first x/W DMAs are still in flight
            warm = tpp.tile([P, P], F32, tag="tp", name="warm")
            for _ in range(40):
                nc.tensor.matmul(warm, ident[:, 0:P], ident, start=True, stop=True)

            # first token tile's x loads go out before W so PE can start as
            # soon as weights land; W is split kv-half then gate-half so the
            # first pair's kv matmuls wait on only half the weight bytes
            xts0 = []
            for k in range(cfg.kt):
                xt = xp.tile([P, cfg.free], DT_BF16, tag="xt", name=f"x0_{k}")
                nc.sync.dma_start(out=xt, in_=xs_d[k * P : (k + 1) * P, 0 : cfg.free])
                xts0.append(xt)
            wt_t = []
            for k in range(cfg.kt):
                w = wts.tile([P, 4 * HD], DT_BF16, tag=f"w{k}", name=f"wt{k}")
                nc.gpsimd.dma_start(out=w, in_=wt_d[k * P : (k + 1) * P, :])
                wt_t.append(w)

            def w_ap(f, k):
                return wt_t[k][:, f * P : (f + 1) * P]

            # comp junction buffers, [d-half partitions, windows]
            comp = [
                cb.tile([P, cfg.nw], F32, tag=f"comp{h}", name=f"comp{h}")
                for h in range(2)
            ]

            # feature tiles: 0,1 = kv chunk0 | 2,3 = kv chunk1
            #                4,5 = gate chunk0 | 6,7 = gate chunk1
            pairs = [(0, 4, "g0a"), (1, 5, "g0b"), (2, 6, "g1a"), (3, 7, "g1b")]
            g_tiles = {nm: {} for _, _, nm in pairs}

            def run_pair(j, fkv, fg, gname):
                pk = mmp.tile([P, cfg.free], F32, tag="mm", name=f"pk{j}_{fkv}")
                for k in range(cfg.kt):
                    nc.tensor.matmul(
                        pk,
                        w_ap(fkv, k),
                        xts[k],
                        start=(k == 0),
                        stop=(k == cfg.kt - 1),
                    )
                pg = mmp.tile([P, cfg.free], F32, tag="mm", name=f"pg{j}_{fg}")
                for k in range(cfg.kt):
                    nc.tensor.matmul(
                        pg,
                        w_ap(fg, k),
                        xts[k],
                        start=(k == 0),
                        stop=(k == cfg.kt - 1),
                    )
                sg = work.tile([P, cfg.free], F32, tag="sig", name=f"sg{j}_{fg}")
                nc.scalar.activation(out=sg, in_=pg, func=AF.Sigmoid)
                # g0 tiles carry a 4-col spill at the end (next tile's first
                # 4 tokens); g1 window reads never cross the tile boundary.
                width = cfg.free + 4 if gname.startswith("g0") else cfg.free
                gt = gp.tile([P, width], F32, tag=gname, name=f"{gname}_{j}")
                nc.vector.tensor_mul(out=gt[:, 0 : cfg.free], in0=pk, in1=sg)
                g_tiles[gname][j] = gt

            def emit_comp_chunk(j, part=None):
                nwin = cfg.chunk_nw if j < cfg.nt - 1 else cfg.last_nw
                col0 = cfg.chunk_nw * j
                for h in range(2):
                    acc = comp[h][:, col0 : col0 + nwin]
                    if part in (None, "g0"):
                        g0 = g_tiles["g0a" if h == 0 else "g0b"][j]
                        for r in range(RATIO):
                            v0 = g0[:, 4 + r : 4 + r + 4 * (nwin - 1) + 1 : 4]
                            s0 = ape_t[:, r * 2 + h : r * 2 + h + 1]
                            if r == 0:
                                nc.vector.tensor_scalar_mul(
                                    out=acc, in0=v0, scalar1=s0
                                )
                            else:
                                nc.vector.scalar_tensor_tensor(
                                    out=acc, in0=v0, scalar=s0, in1=acc,
                                    op0=OP.mult, op1=OP.add,
                                )
                    if part in (None, "g1"):
                        g1 = g_tiles["g1a" if h == 0 else "g1b"][j]
                        for r in range(RATIO):
                            v1 = g1[:, r : r + 4 * (nwin - 1) + 1 : 4]
                            s1 = ape_t[:, 8 + r * 2 + h : 8 + r * 2 + h + 1]
                            nc.vector.scalar_tensor_tensor(
                                out=acc, in0=v1, scalar=s1, in1=acc,
                                op0=OP.mult, op1=OP.add,
                            )

            # ---- phase C: transpose -> RMSNorm -> RoPE -> store ----
            def emit_out_tile(i):
                pta = tpp.tile([P, P], F32, tag="tp", name=f"pta{i}")
                nc.tensor.transpose(pta, comp[0][:, i * P : (i + 1) * P], ident)
                ptb = tpp.tile([P, P], F32, tag="tp", name=f"ptb{i}")
                nc.tensor.transpose(ptb, comp[1][:, i * P : (i + 1) * P], ident)
                yt = work.tile([P, HD], F32, tag="y", name=f"y{i}")
                nc.scalar.copy(out=yt[:, 0:P], in_=pta)
                nc.vector.tensor_copy(out=yt[:, P : 2 * P], in_=ptb)

                sq = work.tile([P, HD], F32, tag="sq", name=f"sq{i}")
                ss = work.tile([P, 1], F32, tag="ss", name=f"ss{i}")
                nc.scalar.activation(
                    out=sq, in_=yt, func=AF.Square, accum_out=ss
                )
                rstd = work.tile([P, 1], F32, tag="rstd", name=f"rstd{i}")
                nc.scalar.activation(
                    out=rstd, in_=ss, func=AF.Sqrt, scale=1.0 / HD, bias=eps_t
                )
                rms = work.tile([P, 1], F32, tag="rms", name=f"rms{i}")
                nc.vector.reciprocal(out=rms, in_=rstd)
                nc.vector.tensor_scalar_mul(out=yt, in0=yt, scalar1=rms)
                nc.vector.tensor_mul(out=yt, in0=yt, in1=nw_t)

                y1 = yt[:, 0:ROPE:2]
                y2 = yt[:, 1:ROPE:2]
                cc = cos_t[:, i, :]
                sn = sin_t[:, i, :]
                m1 = work.tile([P, ROPE // 2], F32, tag="m1", name=f"m1_{i}")
                nc.gpsimd.tensor_mul(out=m1, in0=y1, in1=cc)
                m2 = work.tile([P, ROPE // 2], F32, tag="m2", name=f"m2_{i}")
                nc.gpsimd.tensor_mul(out=m2, in0=y2, in1=sn)
                m3 = work.tile([P, ROPE // 2], F32, tag="m3", name=f"m3_{i}")
                nc.vector.tensor_mul(out=m3, in0=y1, in1=sn)
                m4 = work.tile([P, ROPE // 2], F32, tag="m4", name=f"m4_{i}")
                nc.vector.tensor_mul(out=m4, in0=y2, in1=cc)
                nc.vector.tensor_sub(out=yt[:, 0:ROPE:2], in0=m1, in1=m2)
                nc.vector.tensor_add(out=yt[:, 1:ROPE:2], in0=m3, in1=m4)

                nc.sync.dma_start(out=y_d[i * P : (i + 1) * P, :], in_=yt)

            # ---- phase A + B: projection, gating, windowed compression ----
            for j in range(cfg.nt):
                if j == 0:
                    xts = xts0
                else:
                    xts = []
                    for k in range(cfg.kt):
                        xt = xp.tile(
                            [P, cfg.free], DT_BF16, tag="xt", name=f"x{j}_{k}"
                        )
                        nc.sync.dma_start(
                            out=xt,
                            in_=xs_d[
                                k * P : (k + 1) * P,
                                j * cfg.free : (j + 1) * cfg.free,
                            ],
                        )
                        xts.append(xt)
                run_pair(j, *pairs[0])
                run_pair(j, *pairs[1])
                if j >= 1:
                    for nm in ("g0a", "g0b"):
                        nc.gpsimd.tensor_copy(
                            out=g_tiles[nm][j - 1][:, cfg.free : cfg.free + 4],
                            in_=g_tiles[nm][j][:, 0:4],
                        )
                    # chunk j-1 is fully determined here (its g1 tiles are
                    # from iteration j-1); runs on DVE under pairs 2/3 matmuls
                    emit_comp_chunk(j - 1)
                if j == cfg.nt - 1:
                    # last chunk: g0 part overlaps its own pairs-2/3 matmuls
                    emit_comp_chunk(j, part="g0")
                    # output tiles whose comp chunks are already complete run
                    # under the final pairs' matmuls
                    for i in range(max(0, cfg.wt_tiles - 2)):
                        emit_out_tile(i)
                run_pair(j, *pairs[2])
                run_pair(j, *pairs[3])
            emit_comp_chunk(cfg.nt - 1, part="g1")
            for i in range(max(0, cfg.wt_tiles - 2), cfg.wt_tiles):
                emit_out_tile(i)



    nc.compile()
    return nc


def make_core_inputs(cfg, x_tokens, W, ape, norm_weight, cos_shard, sin_shard):
    """Per-core in_map from natural-layout shards.

    x_tokens: [ntok, dim] bf16 (boundary + own tokens + tail pad, already
    assembled); W: [1024, dim] bf16; ape: [8, 256] f32;
    cos/sin_shard: [nw, 32] f32.
    """
    xs = np.ascontiguousarray(x_tokens.T.astype(BF16, copy=False))
    wt = np.ascontiguousarray(W.T.astype(BF16, copy=False))
    ape2 = np.asarray(ape, np.float32).reshape(2, RATIO, HD)
    ape_sc = np.empty((P, 16), np.float32)
    for c in range(2):
        for r in range(RATIO):
            for h in range(2):
                ape_sc[:, c * 8 + r * 2 + h] = ape2[c, r, h * P : (h + 1) * P]
    nw_b = np.ascontiguousarray(
        np.broadcast_to(np.asarray(norm_weight, np.float32), (P, HD))
    )
    cosp = np.ascontiguousarray(
        np.asarray(cos_shard, np.float32)
        .reshape(cfg.wt_tiles, P, ROPE // 2)
        .transpose(1, 0, 2)
    )
    sinp = np.ascontiguousarray(
        np.asarray(sin_shard, np.float32)
        .reshape(cfg.wt_tiles, P, ROPE // 2)
        .transpose(1, 0, 2)
    )
    return {
        "xs": xs, "wt": wt, "ape": ape_sc, "nw": nw_b, "cosp": cosp, "sinp": sinp
    }


_NC = None
last_results = None


def kernel(**inputs):
    global _NC, last_results
    x = np.asarray(inputs["x"])
    W = np.asarray(inputs["W"])
    ape = np.asarray(inputs["ape"], dtype=np.float32)
    norm_weight = np.asarray(inputs["norm_weight"], dtype=np.float32)
    cos = np.asarray(inputs["cos"], dtype=np.float32)
    sin = np.asarray(inputs["sin"], dtype=np.float32)
    kv_buffer = np.asarray(inputs["kv_buffer"], dtype=np.float32)
    loc = np.asarray(inputs["loc"]).astype(np.int64)

    cfg = FULL
    if _NC is None:
        _NC = build_nc(cfg)

    x_bf = x.astype(BF16, copy=False)
    in_maps = []
    for c in range(N_CORES):
        xtok = np.zeros((cfg.ntok, DIM), BF16)
        lo = TOK * c - RATIO
        s = max(lo, 0)
        e = TOK * c + TOK
        xtok[s - lo : s - lo + (e - s)] = x_bf[s:e]
        in_maps.append(
            make_core_inputs(
                cfg, xtok, W, ape, norm_weight,
                cos[c * NW : (c + 1) * NW], sin[c * NW : (c + 1) * NW],
            )
        )

    res = run_bass_kernel_spmd(_NC, in_maps, core_ids=list(range(N_CORES)))
    last_results = res

    y_full = np.concatenate([res.results[c]["y"] for c in range(N_CORES)], axis=0)
    out = kv_buffer.copy()
    out[loc] = y_full
    return out


# revision 31
# speedup vs baseline: 1.0457x; 1.0141x over previous
"""Trainium2 Bass kernel for nn_Compressor (sparse-attention KV compressor).

Pipeline (data-parallel over 8 NeuronCores, 4096 tokens / 1024 windows each):
  kv_score = x @ W.T (bf16 x bf16 -> fp32)        [tokens, 1024]
  g0/g1    = kv * sigmoid(gate)                    gated chunks, fp32
  comp     = overlapping-window compression        [windows, 256]
  y        = RoPE(RMSNorm(comp))                   [windows, 256]
  out      = kv_buffer with rows[loc] = y          [65536, 256]

Sharding: x is split along the token axis (4096 tokens/core + the 4
boundary tokens of the previous shard for the overlapping chunk); W/ape/
norm_weight are replicated; each core returns its 1024 compressed rows
and the host scatters them into the pool at `loc` (the unshard step).

Device-side layout: x and W are staged transposed ([DIM, tokens] /
[DIM, 1024]) so the matmul runs with the contraction dim on SBUF
partitions and produces kv_score^T tiles [128 features, 456 tokens]
directly. Gating, windowed compression (strided-AP reads, per-partition
ape scalars) and a per-token-tile compression chunk all stay in that
[d, token] layout; only the final [d, w] -> [w, d] flip of the small
compressed output goes through PE transposes. Compression is emitted
per token tile so it pipelines behind the matmul stream instead of
serializing at the end; g0 tiles carry a 4-column spill copied from the
next tile so the stride-4 window reads stay inside one allocation.
"""

import sys

if "/opt/trn_rl_repo" not in sys.path:  # already present under axon boot
    sys.path.insert(0, "/opt/trn_rl_repo")

import numpy as np
import ml_dtypes

import concourse.bass as bass
import concourse.mybir as mybir
import concourse.tile as tile
from concourse import bacc
from concourse.bass_utils import run_bass_kernel_spmd
from concourse.masks import make_identity

BF16 = ml_dtypes.bfloat16
F32 = mybir.dt.float32
DT_BF16 = mybir.dt.bfloat16
AF = mybir.ActivationFunctionType
OP = mybir.AluOpType

P = 128          # SBUF partitions
HD = 256         # compressed head dim
RATIO = 4        # compression stride
ROPE = 64        # rotary dims
EPS = 1e-6
N_CORES = 8

# Full-problem sizes
T = 32768
DIM = 4096
TOK = T // N_CORES        # tokens per core
NW = TOK // RATIO         # windows (compressed tokens) per core
NTOK = TOK + 2 * RATIO    # + 4 boundary tokens in front, + 4 zero pad tail
FREE = 456                # matmul moving free dim; NTOK = 9 * 456
POOL = 65536


class Cfg:
    def __init__(self, dim=DIM, ntok=NTOK, free=FREE, nw=NW, x_bufs=64):
        assert ntok % free == 0
        assert free % RATIO == 0
        assert nw % P == 0
        assert ntok == 4 * nw + 2 * RATIO
        self.dim = dim
        self.ntok = ntok
        self.free = free
        self.nt = ntok // free
        self.kt = dim // P
        self.nw = nw
        self.wt_tiles = nw // P
        self.x_bufs = x_bufs
        self.chunk_nw = free // RATIO            # windows per compression chunk
        self.last_nw = nw - (self.nt - 1) * self.chunk_nw


FULL = Cfg()


def build_nc(cfg=FULL, num_devices=N_CORES):
    nc = bacc.Bacc(
        "TRN2", target_bir_lowering=False, debug=False, num_devices=num_devices
    )
    xs_d = nc.dram_tensor("xs", [cfg.dim, cfg.ntok], DT_BF16, kind="ExternalInput")
    wt_d = nc.dram_tensor("wt", [cfg.dim, 4 * HD], DT_BF16, kind="ExternalInput")
    ape_d = nc.dram_tensor("ape", [P, 16], F32, kind="ExternalInput")
    nw_d = nc.dram_tensor("nw", [P, HD], F32, kind="ExternalInput")
    cos_d = nc.dram_tensor(
        "cosp", [P, cfg.wt_tiles, ROPE // 2], F32, kind="ExternalInput"
    )
    sin_d = nc.dram_tensor(
        "sinp", [P, cfg.wt_tiles, ROPE // 2], F32, kind="ExternalInput"
    )
    y_d = nc.dram_tensor("y", [cfg.nw, HD], F32, kind="ExternalOutput")

    with tile.TileContext(nc) as tc:
        with (
            tc.tile_pool(name="consts", bufs=1) as consts,
            tc.tile_pool(name="wts", bufs=1) as wts,
            tc.tile_pool(name="xp", bufs=cfg.x_bufs) as xp,
            tc.tile_pool(name="gp", bufs=3) as gp,
            tc.tile_pool(name="cb", bufs=1) as cb,
            tc.tile_pool(name="work", bufs=3) as work,
            tc.tile_pool(name="mm", bufs=4, space="PSUM") as mmp,
            tc.tile_pool(name="tp", bufs=4, space="PSUM") as tpp,
        ):
            # ---- constants ----
            ape_t = consts.tile([P, 16], F32, tag="ape")
            nc.sync.dma_start(out=ape_t, in_=ape_d[:, :])
            nw_t = consts.tile([P, HD], F32, tag="nw")
            nc.sync.dma_start(out=nw_t, in_=nw_d[:, :])
            cos_t = consts.tile([P, cfg.wt_tiles, ROPE // 2], F32, tag="cos")
            nc.sync.dma_start(out=cos_t, in_=cos_d[:, :, :])
            sin_t = consts.tile([P, cfg.wt_tiles, ROPE // 2], F32, tag="sin")
            nc.sync.dma_start(out=sin_t, in_=sin_d[:, :, :])
            ident = consts.tile([P, P], F32, tag="id")
            make_identity(nc, ident)
            eps_t = consts.tile([P, 1], F32, tag="eps")
            nc.vector.memset(eps_t, EPS)

            # first token tile's x loads go out before W so PE can start
            # as soon as wt[0] lands; W streams on the gpsimd (SWDGE) queue
            # in parallel with x on the sync (HWDGE) queue
            xts0 = []
            for k in range(cfg.kt):
                xt = xp.tile([P, cfg.free], DT_BF16, tag="xt", name=f"x0_{k}")
                nc.sync.dma_start(out=xt, in_=xs_d[k * P : (k + 1) * P, 0 : cfg.free])
                xts0.append(xt)
            wt_t = []
            for k in range(cfg.kt):
                w = wts.tile([P, 4 * HD], DT_BF16, tag=f"w{k}", name=f"wt{k}")
                nc.gpsimd.dma_start(out=w, in_=wt_d[k * P : (k + 1) * P, :])
                wt_t.append(w)

            def w_ap(f, k):
                return wt_t[k][:, f * P : (f + 1) * P]

            # comp junction buffers, [d-half partitions, windows]
            comp = [
                cb.tile([P, cfg.nw], F32, tag=f"comp{h}", name=f"comp{h}")
                for h in range(2)
            ]

            # feature tiles: 0,1 = kv chunk0 | 2,3 = kv chunk1
            #                4,5 = gate chunk0 | 6,7 = gate chunk1
            pairs = [(0, 4, "g0a"), (1, 5, "g0b"), (2, 6, "g1a"), (3, 7, "g1b")]
            g_tiles = {nm: {} for _, _, nm in pairs}

            def run_pair(j, fkv, fg, gname):
                pk = mmp.tile([P, cfg.free], F32, tag="mm", name=f"pk{j}_{fkv}")
                for k in range(cfg.kt):
                    nc.tensor.matmul(
                        pk,
                        w_ap(fkv, k),
                        xts[k],
                        start=(k == 0),
                        stop=(k == cfg.kt - 1),
                    )
                pg = mmp.tile([P, cfg.free], F32, tag="mm", name=f"pg{j}_{fg}")
                for k in range(cfg.kt):
                    nc.tensor.matmul(
                        pg,
                        w_ap(fg, k),
                        xts[k],
                        start=(k == 0),
                        stop=(k == cfg.kt - 1),
                    )
                sg = work.tile([P, cfg.free], F32, tag="sig", name=f"sg{j}_{fg}")
                nc.scalar.activation(out=sg, in_=pg, func=AF.Sigmoid)
                # g0 tiles carry a 4-col spill at the end (next tile's first
                # 4 tokens); g1 window reads never cross the tile boundary.
                width = cfg.free + 4 if gname.startswith("g0") else cfg.free
                gt = gp.tile([P, width], F32, tag=gname, name=f"{gname}_{j}")
                nc.vector.tensor_mul(out=gt[:, 0 : cfg.free], in0=pk, in1=sg)
                g_tiles[gname][j] = gt

            def emit_comp_chunk(j, part=None):
                nwin = cfg.chunk_nw if j < cfg.nt - 1 else cfg.last_nw
                col0 = cfg.chunk_nw * j
                for h in range(2):
                    acc = comp[h][:, col0 : col0 + nwin]
                    if part in (None, "g0"):
                        g0 = g_tiles["g0a" if h == 0 else "g0b"][j]
                        for r in range(RATIO):
                            v0 = g0[:, 4 + r : 4 + r + 4 * (nwin - 1) + 1 : 4]
                            s0 = ape_t[:, r * 2 + h : r * 2 + h + 1]
                            if r == 0:
                                nc.vector.tensor_scalar_mul(
                                    out=acc, in0=v0, scalar1=s0
                                )
                            else:
                                nc.vector.scalar_tensor_tensor(
                                    out=acc, in0=v0, scalar=s0, in1=acc,
                                    op0=OP.mult, op1=OP.add,
                                )
                    if part in (None, "g1"):
                        g1 = g_tiles["g1a" if h == 0 else "g1b"][j]
                        for r in range(RATIO):
                            v1 = g1[:, r : r + 4 * (nwin - 1) + 1 : 4]
                            s1 = ape_t[:, 8 + r * 2 + h : 8 + r * 2 + h + 1]
                            nc.vector.scalar_tensor_tensor(
                                out=acc, in0=v1, scalar=s1, in1=acc,
                                op0=OP.mult, op1=OP.add,
                            )

            # ---- phase C: transpose -> RMSNorm -> RoPE -> store ----
            def emit_out_tile(i):
                pta = tpp.tile([P, P], F32, tag="tp", name=f"pta{i}")
                nc.tensor.transpose(pta, comp[0][:, i * P : (i + 1) * P], ident)
                ptb = tpp.tile([P, P], F32, tag="tp", name=f"ptb{i}")
                nc.tensor.transpose(ptb, comp[1][:, i * P : (i + 1) * P], ident)
                yt = work.tile([P, HD], F32, tag="y", name=f"y{i}")
                nc.scalar.copy(out=yt[:, 0:P], in_=pta)
                nc.vector.tensor_copy(out=yt[:, P : 2 * P], in_=ptb)

                sq = work.tile([P, HD], F32, tag="sq", name=f"sq{i}")
                ss = work.tile([P, 1], F32, tag="ss", name=f"ss{i}")
                nc.scalar.activation(
                    out=sq, in_=yt, func=AF.Square, accum_out=ss
                )
                rstd = work.tile([P, 1], F32, tag="rstd", name=f"rstd{i}")
                nc.scalar.activation(
                    out=rstd, in_=ss, func=AF.Sqrt, scale=1.0 / HD, bias=eps_t
                )
                rms = work.tile([P, 1], F32, tag="rms", name=f"rms{i}")
                nc.vector.reciprocal(out=rms, in_=rstd)
                nc.vector.tensor_scalar_mul(out=yt, in0=yt, scalar1=rms)
                nc.vector.tensor_mul(out=yt, in0=yt, in1=nw_t)

                y1 = yt[:, 0:ROPE:2]
                y2 = yt[:, 1:ROPE:2]
                cc = cos_t[:, i, :]
                sn = sin_t[:, i, :]
                m1 = work.tile([P, ROPE // 2], F32, tag="m1", name=f"m1_{i}")
                nc.gpsimd.tensor_mul(out=m1, in0=y1, in1=cc)
                m2 = work.tile([P, ROPE // 2], F32, tag="m2", name=f"m2_{i}")
                nc.gpsimd.tensor_mul(out=m2, in0=y2, in1=sn)
                m3 = work.tile([P, ROPE // 2], F32, tag="m3", name=f"m3_{i}")
                nc.vector.tensor_mul(out=m3, in0=y1, in1=sn)
                m4 = work.tile([P, ROPE // 2], F32, tag="m4", name=f"m4_{i}")
                nc.vector.tensor_mul(out=m4, in0=y2, in1=cc)
                nc.vector.tensor_sub(out=yt[:, 0:ROPE:2], in0=m1, in1=m2)
                nc.vector.tensor_add(out=yt[:, 1:ROPE:2], in0=m3, in1=m4)

                nc.sync.dma_start(out=y_d[i * P : (i + 1) * P, :], in_=yt)

            # ---- phase A + B: projection, gating, windowed compression ----
            for j in range(cfg.nt):
                if j == 0:
                    xts = xts0
                else:
                    xts = []
                    for k in range(cfg.kt):
                        xt = xp.tile(
                            [P, cfg.free], DT_BF16, tag="xt", name=f"x{j}_{k}"
                        )
                        nc.sync.dma_start(
                            out=xt,
                            in_=xs_d[
                                k * P : (k + 1) * P,
                                j * cfg.free : (j + 1) * cfg.free,
                            ],
                        )
                        xts.append(xt)
                run_pair(j, *pairs[0])
                run_pair(j, *pairs[1])
                if j >= 1:
                    for nm in ("g0a", "g0b"):
                        nc.gpsimd.tensor_copy(
                            out=g_tiles[nm][j - 1][:, cfg.free : cfg.free + 4],
                            in_=g_tiles[nm][j][:, 0:4],
                        )
                    # chunk j-1 is fully determined here (its g1 tiles are
                    # from iteration j-1); runs on DVE under pairs 2/3 matmuls
                    emit_comp_chunk(j - 1)
                if j == cfg.nt - 1:
                    # last chunk: g0 part overlaps its own pairs-2/3 matmuls
                    emit_comp_chunk(j, part="g0")
                    # output tiles whose comp chunks are already complete run
                    # under the final pairs' matmuls
                    for i in range(max(0, cfg.wt_tiles - 2)):
                        emit_out_tile(i)
                run_pair(j, *pairs[2])
                run_pair(j, *pairs[3])
            emit_comp_chunk(cfg.nt - 1, part="g1")
            for i in range(max(0, cfg.wt_tiles - 2), cfg.wt_tiles):
                emit_out_tile(i)



    nc.compile()
    return nc


def make_core_inputs(cfg, x_tokens, W, ape, norm_weight, cos_shard, sin_shard):
    """Per-core in_map from natural-layout shards.

    x_tokens: [ntok, dim] bf16 (boundary + own tokens + tail pad, already
    assembled); W: [1024, dim] bf16; ape: [8, 256] f32;
    cos/sin_shard: [nw, 32] f32.
    """
    xs = np.ascontiguousarray(x_tokens.T.astype(BF16, copy=False))
    wt = np.ascontiguousarray(W.T.astype(BF16, copy=False))
    ape2 = np.asarray(ape, np.float32).reshape(2, RATIO, HD)
    ape_sc = np.empty((P, 16), np.float32)
    for c in range(2):
        for r in range(RATIO):
            for h in range(2):
                ape_sc[:, c * 8 + r * 2 + h] = ape2[c, r, h * P : (h + 1) * P]
    nw_b = np.ascontiguousarray(
        np.broadcast_to(np.asarray(norm_weight, np.float32), (P, HD))
    )
    cosp = np.ascontiguousarray(
        np.asarray(cos_shard, np.float32)
        .reshape(cfg.wt_tiles, P, ROPE // 2)
        .transpose(1, 0, 2)
    )
    sinp = np.ascontiguousarray(
        np.asarray(sin_shard, np.float32)
        .reshape(cfg.wt_tiles, P, ROPE // 2)
        .transpose(1, 0, 2)
    )
    return {
        "xs": xs, "wt": wt, "ape": ape_sc, "nw": nw_b, "cosp": cosp, "sinp": sinp
    }


_NC = None
last_results = None


def kernel(**inputs):
    global _NC, last_results
    x = np.asarray(inputs["x"])
    W = np.asarray(inputs["W"])
    ape = np.asarray(inputs["ape"], dtype=np.float32)
    norm_weight = np.asarray(inputs["norm_weight"], dtype=np.float32)
    cos = np.asarray(inputs["cos"], dtype=np.float32)
    sin = np.asarray(inputs["sin"], dtype=np.float32)
    kv_buffer = np.asarray(inputs["kv_buffer"], dtype=np.float32)
    loc = np.asarray(inputs["loc"]).astype(np.int64)

    cfg = FULL
    if _NC is None:
        _NC = build_nc(cfg)

    x_bf = x.astype(BF16, copy=False)
    in_maps = []
    for c in range(N_CORES):
        xtok = np.zeros((cfg.ntok, DIM), BF16)
        lo = TOK * c - RATIO
        s = max(lo, 0)
        e = TOK * c + TOK
        xtok[s - lo : s - lo + (e - s)] = x_bf[s:e]
        in_maps.append(
            make_core_inputs(
                cfg, xtok, W, ape, norm_weight,
                cos[c * NW : (c + 1) * NW], sin[c * NW : (c + 1) * NW],
            )
        )

    res = run_bass_kernel_spmd(_NC, in_maps, core_ids=list(range(N_CORES)))
    last_results = res

    y_full = np.concatenate([res.results[c]["y"] for c in range(N_CORES)], axis=0)
    out = kv_buffer.copy()
    out[loc] = y_full
    return out
